# revision 48
# baseline (speedup 1.0000x reference)
"""CapsNet forward kernel for 8 TRN2 NeuronCores (data-parallel over batch).

Per core (b=32 local batch):
  h  = relu(conv(x, conv_w, s1)+cb)            (b,256,20,20)
  u  = squash_8(conv(h, pc_w, s2)+pb)          (b,1152,8)
  routing without materializing u_hat:
    s_k[b,co]   = sum_{p,q} Wc_k[p,q,co] * uT[p,q,b]      (PE)
    v_k         = elementwise-squash(s_k)
    b_upd[r,co] = sum_i Ws[r,c,i] * (1/B sum_b u[b,ri] v[b,co])
    b_ij += AllReduce(b_upd); c = softmax_r(b_ij); Wc = c * W
  Weights (conv_w, conv_b, pc_w, pc_b, W) are identical on every core and
  known at build time, so they are embedded in the NEFF as Const DRAM
  tensors (loaded to HBM once at model-load) instead of per-call inputs.

Partition p (0..127) is an out-channel PAIR (oc=2p / 2p+1), q (0..71) is
(oc%2)*36 + yx; global route r = 9p + q//8, capsule elem i = q%8.
"""
import hashlib

import numpy as np

import concourse.bass as bass
import concourse.mybir as mybir
import concourse.tile as tile
from concourse import bacc
from concourse.bass_utils import run_bass_kernel_spmd
from concourse.masks import make_identity

F32 = mybir.dt.float32
F32R = mybir.dt.float32r
BF16 = mybir.dt.bfloat16
AF = mybir.ActivationFunctionType
ALU = mybir.AluOpType

N_CORES = 8
B = 32              # per-core batch
C1B = 4             # conv1 batch block
PCB = 8             # pc-conv batch block
R, C, O, D = 1152, 10, 16, 8
CO = C * O
Q = 72
RQ = 9
EPS = 1e-5


def _ap(t, offset, dims):
    return bass.AP(t.tensor, t.offset + offset, dims)


def _build(weights, sim_mode=False, reps=1):
    ncores = 1 if sim_mode else N_CORES
    # sim bypasses the AllReduce, so the batch-mean denominator is local-only
    denom = float(B * (1 if sim_mode else N_CORES))
    nc = bacc.Bacc("TRN2", target_bir_lowering=False, debug=False, num_devices=ncores)

    x_d = nc.dram_tensor("x", [B, 1, 28, 28], F32, kind="ExternalInput")
    cw_d = nc.inline_tensor(weights["conv_w"], name="conv_w")
    cb_d = nc.inline_tensor(weights["conv_b"], name="conv_b")
    pw_d = nc.inline_tensor(weights["pc_w"], name="pc_w")
    pb_d = nc.inline_tensor(weights["pc_b"], name="pc_b")
    w_d = nc.inline_tensor(weights["W"], name="W")
    out_d = nc.dram_tensor("out", [B, C, O], F32, kind="ExternalOutput")

    with tile.TileContext(nc) as tc:
        for _rep in range(reps):
            _fwd(nc, tc, x_d, cw_d, cb_d, pw_d, pb_d, w_d, out_d,
                 denom, sim_mode, str(_rep))

    nc.compile()
    return nc


def _fwd(nc, tc, x_d, cw_d, cb_d, pw_d, pb_d, w_d, out_d, denom, sim_mode, sfx):
    with (
        tc.tile_pool(name="persist" + sfx, bufs=1) as pp,
        tc.tile_pool(name="small" + sfx, bufs=1) as sp,
        tc.tile_pool(name="dram" + sfx, bufs=1, space="DRAM") as dp,
    ):
        if True:
            # ---------------- small loads ----------------
            # cw1T partition p = kx*9 + ky (matches xs tile layout below)
            cw1T = pp.tile([81, 256], F32)
            for kx in range(9):
                nc.sync.dma_start(
                    cw1T[kx * 9:(kx + 1) * 9, :],
                    bass.AP(cw_d, kx, [[9, 9], [81, 256]]),
                )
            cw1bf = pp.tile([81, 256], BF16)
            nc.vector.tensor_copy(cw1bf[:], cw1T[:])
            cb0 = sp.tile([128, 1], F32)
            cb1 = sp.tile([128, 1], F32)
            cbv = cb_d.ap().rearrange("(a b) -> a b", b=1)
            nc.sync.dma_start(cb0[:], cbv[0:128])
            nc.sync.dma_start(cb1[:], cbv[128:256])
            pcb0 = sp.tile([128, 1], F32)
            pcb1 = sp.tile([128, 1], F32)
            pbv = pb_d.ap().rearrange("(p two) -> p two", two=2)
            nc.sync.dma_start(pcb0[:], pbv[:, 0:1])
            nc.sync.dma_start(pcb1[:], pbv[:, 1:2])
            ident128 = pp.tile([128, 128], BF16)
            make_identity(nc, ident128[:])
            # x -> bf16, padded to 792/image, staged in DRAM; then one
            # HBM->HBM 4D im2col so the SBUF load is a single full-width DMA
            with tc.tile_pool(name="xprep", bufs=1) as xp:
                xsb = xp.tile([B, 784], F32)
                nc.sync.dma_start(
                    xsb[:], x_d.ap().rearrange("b one h w -> b (one h w)"))
                xbf = xp.tile([B, 792], BF16)
                nc.vector.memset(xbf[:, 784:], 0.0)
                nc.vector.tensor_copy(xbf[:, 0:784], xsb[:])
                xpad_d = dp.tile([B, 792], BF16, tag="xpad" + sfx)
                nc.sync.dma_start(xpad_d[:], xbf[:])
            xs2_d = dp.tile([81, B, 560], BF16, tag="xs2" + sfx)
            for kx in range(9):
                nc.sync.dma_start(
                    _ap(xs2_d, kx * 9 * B * 560,
                        [[B * 560, 9], [560, B], [1, 560]]),
                    _ap(xpad_d, kx, [[28, 9], [792, B], [1, 560]]),
                )

            h_sb = [pp.tile([128, B, 20, 20], BF16, name=f"h{i}_" + sfx) for i in range(2)]
            uTpre = pp.tile([128, Q, B], F32)
            uT = pp.tile([128, Q, B], BF16)

            ws2 = pp.tile([128, Q, C], F32)

            # -------- conv1 + primary-caps conv (pc_w ic0 loads hoisted) -----
            with tc.tile_pool(name="pcw", bufs=1) as pwp:
                def load_pwin(ic_t, par):
                    pwin = pwp.tile([128, 128, 81], BF16, tag="pwin", bufs=2)
                    for sub in range(4):
                        tmp = pwp.tile([128, 32, 81], F32, tag="pwt", bufs=1)
                        src = bass.AP(
                            pw_d,
                            (2 * sub * 32 + par) * 20736 + ic_t * 128 * 81,
                            [[81, 128], [2 * 20736, 32], [1, 81]],
                        )
                        nc.sync.dma_start(tmp[:], src)
                        eng = nc.vector if sub % 2 == 0 else nc.gpsimd
                        eng.tensor_copy(
                            pwin[:, sub * 32:(sub + 1) * 32, :], tmp[:])
                    return pwin

                with (
                    tc.tile_pool(name="c1in", bufs=1) as c1p,
                    tc.tile_pool(name="c1ps", bufs=1, space="PSUM") as c1ps,
                ):
                    # partition p = kx*9 + ky; one contiguous full-width load,
                    # queued BEFORE the pc_w loads so conv1 starts early
                    xs_all = c1p.tile([81, B, 560], BF16, tag="xsall")
                    nc.sync.dma_start(
                        xs_all[:].rearrange("p b c -> p (b c)"),
                        _ap(xs2_d, 0, [[B * 560, 81], [1, B * 560]]),
                    )
                    pwins = {(0, 0): load_pwin(0, 0), (0, 1): load_pwin(0, 1)}
                    for bb in range(B):
                        for half in range(2):
                            for oct_ in range(2):
                                ps = c1ps.tile([128, 280], F32, tag="c1", bufs=7)
                                lhsT = cw1bf[:, oct_ * 128:(oct_ + 1) * 128]
                                rhs = xs_all[:, bb, half * 280: half * 280 + 280]
                                nc.tensor.matmul(ps[:], lhsT, rhs, start=True, stop=True)
                                hd = h_sb[oct_][:, bb, half * 10:(half + 1) * 10, :]
                                pv = ps[:].rearrange("p (y x) -> p y x", x=28)[:, :, 0:20]
                                cbx = (cb0 if oct_ == 0 else cb1)
                                if (bb * 4 + half * 2 + oct_) % 2 == 0:
                                    nc.scalar.activation(
                                        hd, pv, AF.Relu, bias=cbx[:], scale=1.0)
                                else:
                                    # bias-add + relu on DVE to split the
                                    # epilogue across both engines
                                    nc.vector.tensor_scalar(
                                        hd, pv, cbx[:], 0.0,
                                        op0=ALU.add, op1=ALU.max)

                pcps_ctx = tc.tile_pool(name="pcps", bufs=1, space="PSUM")
                pcps = pcps_ctx.__enter__()
                psums = {}
                for blk in range(B // PCB):
                    for par in range(2):
                        psums[(blk, par)] = pcps.tile(
                            [128, PCB, 36], F32, tag=f"pc{blk}{par}", bufs=1,
                            name=f"pcps{blk}{par}_" + sfx,
                        )
                for ic_t in range(2):
                    for par in range(2):
                        pwin = pwins.get((ic_t, par))
                        if pwin is None:
                            pwin = load_pwin(ic_t, par)
                        for t in range(81):
                            ky, kx = t // 9, t % 9
                            for blk in range(B // PCB):
                                rhs = h_sb[ic_t][:, blk * PCB:(blk + 1) * PCB,
                                                 ky:ky + 12:2, kx:kx + 12:2]
                                nc.tensor.matmul(
                                    psums[(blk, par)][:], pwin[:, :, t], rhs,
                                    start=(ic_t == 0 and t == 0),
                                    stop=(ic_t == 1 and t == 80),
                                )
                # One-time W pre-stage (issued here so the W DMAs queue
                # behind the pc_w loads and the DVE transforms run during the
                # pc-conv matmuls instead of blocking the routing phase).
                wpre = pp.tile([128, RQ, D, C, O], BF16, name="wpre" + sfx)
                for rq in range(RQ):
                    wtmp = pwp.tile([128, C, O, D], F32, tag="wtmp", bufs=1)
                    nc.sync.dma_start(
                        wtmp[:],
                        bass.AP(w_d, rq * 1280,
                                [[RQ * 1280, 128], [128, C], [8, O], [1, D]]),
                    )
                    nc.vector.tensor_copy(
                        wpre[:, rq],
                        wtmp[:].rearrange("p c o i -> p i c o"),
                    )
                    nc.vector.tensor_reduce(
                        ws2[:, rq * D:(rq + 1) * D, :],
                        wtmp[:].rearrange("p c o i -> p i c o"),
                        axis=mybir.AxisListType.X, op=ALU.add,
                    )

                for blk in range(B // PCB):
                    for par in range(2):
                        nc.scalar.activation(
                            uTpre[:, par * 36:(par + 1) * 36, blk * PCB:(blk + 1) * PCB]
                            .rearrange("p q b -> p b q"),
                            psums[(blk, par)][:],
                            AF.Identity,
                            bias=(pcb0 if par == 0 else pcb1)[:],
                            scale=1.0,
                        )
                pcps_ctx.__exit__(None, None, None)

            # ---------------- squash over capsule dim ----------------
            sq = sp.tile([128, Q, B], F32)
            nc.vector.tensor_tensor(sq[:], uTpre[:], uTpre[:], op=ALU.mult)
            sn = sp.tile([128, RQ, B], F32)
            nc.vector.tensor_reduce(
                sn[:], sq[:].rearrange("p (rq i) b -> p rq b i", i=D),
                axis=mybir.AxisListType.X, op=ALU.add,
            )
            t1 = sp.tile([128, RQ, B], F32)
            nc.vector.tensor_scalar_add(t1[:], sn[:], 1.0)
            t2 = sp.tile([128, RQ, B], F32)
            nc.scalar.activation(t2[:], sn[:], AF.Sqrt)
            nc.vector.tensor_scalar_add(t2[:], t2[:], EPS)
            nc.vector.tensor_tensor(t1[:], t1[:], t2[:], op=ALU.mult)
            t3 = sp.tile([128, RQ, B], F32)
            nc.vector.reciprocal(t3[:], t1[:])
            nc.vector.tensor_tensor(t3[:], sn[:], t3[:], op=ALU.mult)  # coef
            nc.vector.tensor_tensor(
                uT[:].rearrange("p (rq i) b -> p rq i b", i=D),
                uTpre[:].rearrange("p (rq i) b -> p rq i b", i=D),
                _ap(t3[:], 0, [t3[:].ap[0], [B, RQ], [0, D], [1, B]]),
                op=ALU.mult,
            )

            # ---------------- routing ----------------
            with (
                tc.tile_pool(name="route", bufs=1) as rp,
                tc.tile_pool(name="rps", bufs=1, space="PSUM") as rps,
            ):
                bij = [rp.tile([80, RQ, 128], F32, name=f"bij{i}_" + sfx)
                       for i in range(2)]
                # u2[b, q, p] = u[b, 9p+q//8, q%8]
                u2 = rp.tile([B, Q, 128], BF16, name="u2_" + sfx)
                bupd = rp.tile([128, RQ, CO], BF16, name="bupd" + sfx)
                # u2[b,q,p] = uT[p,q,b] via PE transposes
                for q in range(Q):
                    tps = rps.tile([B, 128], BF16, tag="ut", bufs=2)
                    nc.tensor.transpose(tps[:], uT[:, q, :], ident128[:])
                    nc.scalar.copy(u2[:, q, :], tps[:])

                for it in range(3):
                    cbf = []
                    if it > 0:
                        for hf in range(2):
                            bt = bij[hf]
                            btf = bt[:].rearrange("co rq p -> co (rq p)")
                            mx = rp.tile([80, 1], F32, tag="mx", bufs=4)
                            nc.vector.tensor_reduce(
                                mx[:], btf, axis=mybir.AxisListType.X,
                                op=ALU.max, negate=True,
                            )
                            cb_t = rp.tile([80, RQ, 128], BF16, tag=f"c{hf}",
                                           bufs=2)
                            sm = rp.tile([80, 1], F32, tag="sm", bufs=4)
                            nc.scalar.activation(
                                cb_t[:].rearrange("co rq p -> co (rq p)"), btf,
                                AF.Exp, bias=mx[:], scale=1.0,
                                accum_out=sm[:],
                            )
                            rc = rp.tile([80, 1], F32, tag="rc", bufs=4)
                            nc.vector.reciprocal(rc[:], sm[:])
                            nc.vector.tensor_scalar_mul(
                                cb_t[:].rearrange("co rq p -> co (rq p)"),
                                cb_t[:].rearrange("co rq p -> co (rq p)"), rc[:])
                            cbf.append(cb_t)

                    # s matmuls; Wc built from pre-staged wpre (it>0 only)
                    sps = rps.tile([B, CO], F32, tag="s", bufs=1)
                    for rq in range(RQ):
                        if it == 0:
                            wmm = wpre[:, rq]
                        else:
                            wcs = rp.tile([128, D, C, O], BF16, tag="wcs", bufs=3)
                            for hf in range(2):
                                ctp = rps.tile([128, 80], BF16, tag="ct", bufs=1)
                                nc.tensor.transpose(
                                    ctp[:], cbf[hf][:, rq, :], ident128[:80, :80]
                                )
                                nc.vector.tensor_tensor(
                                    wcs[:, :, hf * 5:(hf + 1) * 5, :],
                                    _ap(ctp[:], 0,
                                        [ctp[:].ap[0], [0, D], [16, C // 2], [1, O]]),
                                    wpre[:, rq, :, hf * 5:(hf + 1) * 5, :],
                                    op=ALU.mult,
                                )
                            wmm = wcs[:]
                        for i in range(D):
                            q = rq * D + i
                            nc.tensor.matmul(
                                sps[:], uT[:, q, :],
                                wmm[:, i].rearrange("p c o -> p (c o)"),
                                start=(q == 0), stop=(q == Q - 1),
                            )

                    ssb = rp.tile([B, CO], F32, tag="ssb", bufs=2)
                    nc.scalar.activation(
                        ssb[:], sps[:], AF.Copy,
                        scale=(1.0 / R) if it == 0 else 1.0,
                    )
                    # elementwise squash -> v
                    sa = rp.tile([B, CO], F32, tag="sa", bufs=2)
                    nc.vector.tensor_tensor(sa[:], ssb[:], ssb[:], op=ALU.mult)
                    sb_ = rp.tile([B, CO], F32, tag="sb_", bufs=2)
                    nc.scalar.activation(sb_[:], sa[:], AF.Sqrt)
                    nc.vector.tensor_scalar_add(sb_[:], sb_[:], EPS)
                    sc_ = rp.tile([B, CO], F32, tag="sc_", bufs=2)
                    nc.vector.tensor_scalar_add(sc_[:], sa[:], 1.0)
                    nc.vector.tensor_tensor(sb_[:], sb_[:], sc_[:], op=ALU.mult)
                    nc.vector.reciprocal(sb_[:], sb_[:])
                    nc.vector.tensor_tensor(sa[:], sa[:], sb_[:], op=ALU.mult)  # coef
                    if it == 2:
                        vout = rp.tile([B, CO], F32, tag="vout")
                        nc.vector.tensor_tensor(vout[:], ssb[:], sa[:], op=ALU.mult)
                        nc.sync.dma_start(
                            out_d.ap().rearrange("b c o -> b (c o)"), vout[:]
                        )
                    else:
                        vbf = rp.tile([B, CO], BF16, tag="vbf", bufs=2)
                        nc.vector.tensor_tensor(vbf[:], ssb[:], sa[:], op=ALU.mult)
                        for rq in range(RQ):
                            # G in 3 bank-sized psum pieces (matmul outputs
                            # must not cross the 2KB psum bank boundary)
                            pieces = [(0, 3), (3, 3), (6, 2)]
                            gtiles = []
                            for k, (_, n) in enumerate(pieces):
                                gt = rps.tile([128, n, CO], F32, tag=f"g{k}",
                                              bufs=1, name=f"gps{k}_" + sfx)
                                gtiles.append(gt)
                            for i in range(D):
                                q = rq * D + i
                                k = 0 if i < 3 else (1 if i < 6 else 2)
                                i0 = pieces[k][0]
                                nc.tensor.matmul(
                                    gtiles[k][:, i - i0, :],
                                    u2[:, q, :], vbf[:],
                                    start=True, stop=True,
                                )
                            # gw[p,i,c,o] = G[p,i,c,o]/denom * ws2[p,(rq,i),c]
                            gw = rp.tile([128, D, C, O], F32, tag="gw", bufs=2)
                            for k, (i0, n) in enumerate(pieces):
                                ws_bc = _ap(
                                    ws2[:, rq * D + i0, :], 0,
                                    [ws2[:, 0, :].ap[0], [C, n], [1, C], [0, O]])
                                nc.vector.scalar_tensor_tensor(
                                    gw[:, i0:i0 + n],
                                    gtiles[k][:].rearrange(
                                        "p i (c o) -> p i c o", o=O),
                                    1.0 / denom, ws_bc,
                                    op0=ALU.mult, op1=ALU.mult,
                                )
                            # sum over i via contiguous binary tree (gpsimd,
                            # freeing DVE for the next rq's gw multiply)
                            gh = rp.tile([128, 4, C, O], F32, tag="gh", bufs=2)
                            nc.gpsimd.tensor_tensor(
                                gh[:], gw[:, 0:4], gw[:, 4:8], op=ALU.add)
                            gq = rp.tile([128, 2, C, O], F32, tag="gq", bufs=2)
                            nc.gpsimd.tensor_tensor(
                                gq[:], gh[:, 0:2], gh[:, 2:4], op=ALU.add)
                            nc.gpsimd.tensor_tensor(
                                bupd[:, rq, :].rearrange("p (c o) -> p c o", o=O),
                                gq[:, 0], gq[:, 1], op=ALU.add)
                        arin = dp.tile([128, RQ * CO], BF16,
                                       tag=f"arin{it}_" + sfx)
                        arout = dp.tile([128, RQ * CO], BF16,
                                        tag=f"arout{it}_" + sfx,
                                        addr_space="Shared")
                        nc.sync.dma_start(
                            arin, bupd[:].rearrange("p rq co -> p (rq co)")
                        )
                        if sim_mode:
                            nc.sync.dma_start(arout, arin)
                        else:
                            nc.gpsimd.collective_compute(
                                "AllReduce", ALU.add,
                                replica_groups=[list(range(N_CORES))],
                                ins=[arin.opt()], outs=[arout.opt()],
                            )
                        art = rp.tile([128, RQ, CO], BF16, tag="art", bufs=2)
                        nc.sync.dma_start(
                            art[:].rearrange("p rq co -> p (rq co)"), arout
                        )
                        # transpose (p,(rq,co)) -> bij[co, r=9p+rq] on PE
                        for hf in range(2):
                            for rq in range(RQ):
                                btp = rps.tile([80, 128], BF16, tag="bt", bufs=1)
                                nc.tensor.transpose(
                                    btp[:], art[:, rq, hf * 80:(hf + 1) * 80],
                                    ident128[:],
                                )
                                if it == 0:
                                    nc.vector.tensor_copy(bij[hf][:, rq, :], btp[:])
                                else:
                                    nc.vector.tensor_tensor(
                                        bij[hf][:, rq, :], bij[hf][:, rq, :],
                                        btp[:], op=ALU.add,
                                    )

_NC = None
_WHASH = None


def _weights(inputs):
    return {
        k: np.ascontiguousarray(inputs[k], dtype=np.float32)
        for k in ("conv_w", "conv_b", "pc_w", "pc_b", "W")
    }


def _whash(w):
    h = hashlib.sha256()
    for k in ("conv_w", "conv_b", "pc_w", "pc_b", "W"):
        h.update(w[k].tobytes())
    return h.hexdigest()


def _ensure_built(inputs):
    global _NC, _WHASH
    w = _weights(inputs)
    hsh = _whash(w)
    if _NC is None or hsh != _WHASH:
        _NC = _build(w)
        _WHASH = hsh
    return _NC


def _in_maps(inputs):
    x = np.ascontiguousarray(inputs["x"], dtype=np.float32)
    return [{"x": x[c * B:(c + 1) * B]} for c in range(N_CORES)]


def kernel(**inputs):
    nc = _ensure_built(inputs)
    res = run_bass_kernel_spmd(nc, _in_maps(inputs), core_ids=list(range(N_CORES)))
    return np.concatenate([res.results[c]["out"] for c in range(N_CORES)], axis=0)


def _pipelined_mins(nc, in_maps, ns, n_trials):
    import time
    import jax
    from jax.sharding import Mesh, PartitionSpec
    from jax.experimental.shard_map import shard_map
    import concourse.bass2jax as b2j
    import concourse.mybir as mybir_

    b2j.install_neuronx_cc_hook()
    partition_name = nc.partition_id_tensor.name if nc.partition_id_tensor else None
    in_names, out_names, out_avals, zero_outs = [], [], [], []
    for alloc in nc.m.functions[0].allocations:
        if not isinstance(alloc, mybir_.MemoryLocationSet):
            continue
        name = alloc.memorylocations[0].name
        if alloc.kind == "ExternalInput":
            if name != partition_name:
                in_names.append(name)
        elif alloc.kind == "ExternalOutput":
            shape = tuple(alloc.tensor_shape)
            dtype = mybir_.dt.np(alloc.dtype)
            out_names.append(name)
            out_avals.append(jax.core.ShapedArray(shape, dtype))
            zero_outs.append(np.zeros(shape, dtype))
    n_params = len(in_names)
    n_outs = len(out_avals)
    all_in_names = list(in_names) + out_names
    if partition_name is not None:
        all_in_names.append(partition_name)
    donate = tuple(range(n_params, n_params + n_outs))

    def _body(*args):
        operands = list(args)
        if partition_name is not None:
            operands.append(b2j.partition_id_tensor())
        outs = b2j._bass_exec_p.bind(
            *operands,
            out_avals=tuple(out_avals),
            in_names=tuple(all_in_names),
            out_names=tuple(out_names),
            lowering_input_output_aliases=(),
            sim_require_finite=True,
            sim_require_nnan=True,
            nc=nc,
        )
        return tuple(outs)

    devices = jax.devices()[:N_CORES]
    mesh = Mesh(np.asarray(devices), ("core",))
    in_specs = (PartitionSpec("core"),) * (n_params + n_outs)
    out_specs = (PartitionSpec("core"),) * n_outs
    sharded = jax.jit(
        shard_map(_body, mesh=mesh, in_specs=in_specs, out_specs=out_specs,
                  check_rep=False),
        donate_argnums=donate, keep_unused=True,
    )
    concat_in = [
        jax.device_put(
            np.concatenate([np.asarray(in_maps[c][n]) for c in range(N_CORES)],
                           axis=0))
        for n in in_names
    ]
    mins = {}
    for n in ns:
        times = []
        for _ in range(n_trials):
            zsets = [
                [jax.device_put(
                    np.zeros((N_CORES * z.shape[0], *z.shape[1:]), z.dtype))
                 for z in zero_outs]
                for _ in range(n)
            ]
            jax.block_until_ready(zsets)
            t0 = time.perf_counter()
            outs = [sharded(*concat_in, *zsets[k]) for k in range(n)]
            jax.block_until_ready(outs)
            times.append((time.perf_counter() - t0) * 1e9)
        mins[n] = min(times)
        print(f"  N={n} pipelined dispatches, wall (ms): "
              + ", ".join(f"{t/1e6:.2f}" for t in times))
    return mins


_TIMING_REPS = 8


def run_timed(**inputs):
    """Measure per-forward HW time of the compiled kernel.

    The axon NTFF profiling hook is unavailable here, so there is no
    hardware profile to read. Wall-clock around one dispatch is useless:
    a jitted `x+1` costs the same ~60 ms tunnel round-trip, and even the
    marginal cost of an extra pipelined dispatch is ~3-4 ms of dispatch
    overhead for an empty kernel. Instead, build a second NEFF whose body
    is the SAME forward pass replicated _TIMING_REPS times back-to-back,
    measure the marginal cost per pipelined dispatch of both NEFFs, and
    difference them:

        slope(K) - slope(1) = (K - 1) * t_body

    which cancels both the tunnel round-trip and the per-dispatch
    overhead, leaving the steady-state device-side time of one forward
    (conv + routing + 2 AllReduces across the 8 cores).
    """
    nc1 = _ensure_built(inputs)
    ncK = _build(_weights(inputs), reps=_TIMING_REPS)
    in_maps = _in_maps(inputs)

    ns = (1, 48)
    n1, n2 = ns
    print(f"[timing] reps=1 NEFF:")
    m1 = _pipelined_mins(nc1, in_maps, ns, n_trials=12)
    print(f"[timing] reps={_TIMING_REPS} NEFF:")
    mK = _pipelined_mins(ncK, in_maps, ns, n_trials=12)
    slope1 = (m1[n2] - m1[n1]) / (n2 - n1)
    slopeK = (mK[n2] - mK[n1]) / (n2 - n1)
    body = (slopeK - slope1) / (_TIMING_REPS - 1)
    print(f"single-dispatch wall: {m1[n1]/1e6:.3f} ms; "
          f"marginal/dispatch reps=1: {slope1/1e6:.4f} ms; "
          f"reps={_TIMING_REPS}: {slopeK/1e6:.4f} ms; "
          f"per-forward body: {body/1e6:.4f} ms")
    return int(max(body, 1.0))


# revision 52
# speedup vs baseline: 1.0103x; 1.0103x over previous
"""CapsNet forward kernel for 8 TRN2 NeuronCores (data-parallel over batch).

Per core (b=32 local batch):
  h  = relu(conv(x, conv_w, s1)+cb)            (b,256,20,20)
  u  = squash_8(conv(h, pc_w, s2)+pb)          (b,1152,8)
  routing without materializing u_hat:
    s_k[b,co]   = sum_{p,q} Wc_k[p,q,co] * uT[p,q,b]      (PE)
    v_k         = elementwise-squash(s_k)
    b_upd[r,co] = sum_i Ws[r,c,i] * (1/B sum_b u[b,ri] v[b,co])
    b_ij += AllReduce(b_upd); c = softmax_r(b_ij); Wc = c * W
  Weights (conv_w, conv_b, pc_w, pc_b, W) are identical on every core and
  known at build time, so they are embedded in the NEFF as Const DRAM
  tensors (loaded to HBM once at model-load) instead of per-call inputs.

Partition p (0..127) is an out-channel PAIR (oc=2p / 2p+1), q (0..71) is
(oc%2)*36 + yx; global route r = 9p + q//8, capsule elem i = q%8.
"""
import hashlib

import numpy as np

import concourse.bass as bass
import concourse.mybir as mybir
import concourse.tile as tile
from concourse import bacc
from concourse.bass_utils import run_bass_kernel_spmd
from concourse.masks import make_identity

F32 = mybir.dt.float32
F32R = mybir.dt.float32r
BF16 = mybir.dt.bfloat16
AF = mybir.ActivationFunctionType
ALU = mybir.AluOpType

N_CORES = 8
B = 32              # per-core batch
C1B = 4             # conv1 batch block
PCB = 8             # pc-conv batch block
R, C, O, D = 1152, 10, 16, 8
CO = C * O
Q = 72
RQ = 9
EPS = 1e-5


def _ap(t, offset, dims):
    return bass.AP(t.tensor, t.offset + offset, dims)


def _build(weights, sim_mode=False, reps=1):
    ncores = 1 if sim_mode else N_CORES
    # sim bypasses the AllReduce, so the batch-mean denominator is local-only
    denom = float(B * (1 if sim_mode else N_CORES))
    nc = bacc.Bacc("TRN2", target_bir_lowering=False, debug=False, num_devices=ncores)

    x_d = nc.dram_tensor("x", [B, 1, 28, 28], F32, kind="ExternalInput")
    cw_d = nc.inline_tensor(weights["conv_w"], name="conv_w")
    cb_d = nc.inline_tensor(weights["conv_b"], name="conv_b")
    pw_d = nc.inline_tensor(weights["pc_w"], name="pc_w")
    pb_d = nc.inline_tensor(weights["pc_b"], name="pc_b")
    w_d = nc.inline_tensor(weights["W"], name="W")
    out_d = nc.dram_tensor("out", [B, C, O], F32, kind="ExternalOutput")

    with tile.TileContext(nc) as tc:
        for _rep in range(reps):
            _fwd(nc, tc, x_d, cw_d, cb_d, pw_d, pb_d, w_d, out_d,
                 denom, sim_mode, str(_rep))

    nc.compile()
    return nc


def _fwd(nc, tc, x_d, cw_d, cb_d, pw_d, pb_d, w_d, out_d, denom, sim_mode, sfx):
    with (
        tc.tile_pool(name="persist" + sfx, bufs=1) as pp,
        tc.tile_pool(name="small" + sfx, bufs=1) as sp,
        tc.tile_pool(name="dram" + sfx, bufs=1, space="DRAM") as dp,
    ):
        if True:
            # ---------------- small loads ----------------
            # cw1T partition p = kx*9 + ky (matches xs tile layout below)
            cw1T = pp.tile([81, 256], F32)
            for kx in range(9):
                nc.sync.dma_start(
                    cw1T[kx * 9:(kx + 1) * 9, :],
                    bass.AP(cw_d, kx, [[9, 9], [81, 256]]),
                )
            cw1bf = pp.tile([81, 256], BF16)
            nc.vector.tensor_copy(cw1bf[:], cw1T[:])
            cb0 = sp.tile([128, 1], F32)
            cb1 = sp.tile([128, 1], F32)
            cbv = cb_d.ap().rearrange("(a b) -> a b", b=1)
            nc.sync.dma_start(cb0[:], cbv[0:128])
            nc.sync.dma_start(cb1[:], cbv[128:256])
            pcb0 = sp.tile([128, 1], F32)
            pcb1 = sp.tile([128, 1], F32)
            pbv = pb_d.ap().rearrange("(p two) -> p two", two=2)
            nc.sync.dma_start(pcb0[:], pbv[:, 0:1])
            nc.sync.dma_start(pcb1[:], pbv[:, 1:2])
            ident128 = pp.tile([128, 128], BF16)
            make_identity(nc, ident128[:])
            # x -> bf16, padded to 792/image, staged in DRAM; then one
            # HBM->HBM 4D im2col so the SBUF load is a single full-width DMA
            with tc.tile_pool(name="xprep", bufs=1) as xp:
                xsb = xp.tile([B, 784], F32)
                nc.sync.dma_start(
                    xsb[:], x_d.ap().rearrange("b one h w -> b (one h w)"))
                xbf = xp.tile([B, 792], BF16)
                nc.vector.memset(xbf[:, 784:], 0.0)
                nc.vector.tensor_copy(xbf[:, 0:784], xsb[:])
                xpad_d = dp.tile([B, 792], BF16, tag="xpad" + sfx)
                nc.sync.dma_start(xpad_d[:], xbf[:])
            xs2_d = dp.tile([81, B, 560], BF16, tag="xs2" + sfx)
            for kx in range(9):
                nc.sync.dma_start(
                    _ap(xs2_d, kx * 9 * B * 560,
                        [[B * 560, 9], [560, B], [1, 560]]),
                    _ap(xpad_d, kx, [[28, 9], [792, B], [1, 560]]),
                )

            h_sb = [pp.tile([128, B, 20, 20], BF16, name=f"h{i}_" + sfx) for i in range(2)]
            uTpre = pp.tile([128, Q, B], F32)
            uT = pp.tile([128, Q, B], BF16)

            ws2 = pp.tile([128, Q, C], F32)

            # -------- conv1 + primary-caps conv (pc_w ic0 loads hoisted) -----
            with tc.tile_pool(name="pcw", bufs=1) as pwp:
                def load_pwin(ic_t, par):
                    pwin = pwp.tile([128, 128, 81], BF16, tag="pwin", bufs=2)
                    for sub in range(4):
                        tmp = pwp.tile([128, 32, 81], F32, tag="pwt", bufs=2)
                        src = bass.AP(
                            pw_d,
                            (2 * sub * 32 + par) * 20736 + ic_t * 128 * 81,
                            [[81, 128], [2 * 20736, 32], [1, 81]],
                        )
                        nc.sync.dma_start(tmp[:], src)
                        eng = nc.vector if sub % 2 == 0 else nc.gpsimd
                        eng.tensor_copy(
                            pwin[:, sub * 32:(sub + 1) * 32, :], tmp[:])
                    return pwin

                with (
                    tc.tile_pool(name="c1in", bufs=1) as c1p,
                    tc.tile_pool(name="c1ps", bufs=1, space="PSUM") as c1ps,
                ):
                    # partition p = kx*9 + ky; one contiguous full-width load,
                    # queued BEFORE the pc_w loads so conv1 starts early
                    xs_all = c1p.tile([81, B, 560], BF16, tag="xsall")
                    nc.sync.dma_start(
                        xs_all[:].rearrange("p b c -> p (b c)"),
                        _ap(xs2_d, 0, [[B * 560, 81], [1, B * 560]]),
                    )
                    pwins = {(0, 0): load_pwin(0, 0), (0, 1): load_pwin(0, 1)}
                    for bb in range(B):
                        for half in range(2):
                            for oct_ in range(2):
                                ps = c1ps.tile([128, 280], F32, tag="c1", bufs=7)
                                lhsT = cw1bf[:, oct_ * 128:(oct_ + 1) * 128]
                                rhs = xs_all[:, bb, half * 280: half * 280 + 280]
                                nc.tensor.matmul(ps[:], lhsT, rhs, start=True, stop=True)
                                hd = h_sb[oct_][:, bb, half * 10:(half + 1) * 10, :]
                                pv = ps[:].rearrange("p (y x) -> p y x", x=28)[:, :, 0:20]
                                cbx = (cb0 if oct_ == 0 else cb1)
                                if (bb * 4 + half * 2 + oct_) % 2 == 0:
                                    nc.scalar.activation(
                                        hd, pv, AF.Relu, bias=cbx[:], scale=1.0)
                                else:
                                    # bias-add + relu on DVE to split the
                                    # epilogue across both engines
                                    nc.vector.tensor_scalar(
                                        hd, pv, cbx[:], 0.0,
                                        op0=ALU.add, op1=ALU.max)

                pcps_ctx = tc.tile_pool(name="pcps", bufs=1, space="PSUM")
                pcps = pcps_ctx.__enter__()
                psums = {}
                for blk in range(B // PCB):
                    for par in range(2):
                        psums[(blk, par)] = pcps.tile(
                            [128, PCB, 36], F32, tag=f"pc{blk}{par}", bufs=1,
                            name=f"pcps{blk}{par}_" + sfx,
                        )
                for ic_t in range(2):
                    for par in range(2):
                        pwin = pwins.get((ic_t, par))
                        if pwin is None:
                            pwin = load_pwin(ic_t, par)
                        for t in range(81):
                            ky, kx = t // 9, t % 9
                            for blk in range(B // PCB):
                                rhs = h_sb[ic_t][:, blk * PCB:(blk + 1) * PCB,
                                                 ky:ky + 12:2, kx:kx + 12:2]
                                nc.tensor.matmul(
                                    psums[(blk, par)][:], pwin[:, :, t], rhs,
                                    start=(ic_t == 0 and t == 0),
                                    stop=(ic_t == 1 and t == 80),
                                )
                # One-time W pre-stage (issued here so the W DMAs queue
                # behind the pc_w loads and the DVE transforms run during the
                # pc-conv matmuls instead of blocking the routing phase).
                wpre = pp.tile([128, RQ, D, C, O], BF16, name="wpre" + sfx)
                for rq in range(RQ):
                    wtmp = pwp.tile([128, C, O, D], F32, tag="wtmp", bufs=1)
                    nc.sync.dma_start(
                        wtmp[:],
                        bass.AP(w_d, rq * 1280,
                                [[RQ * 1280, 128], [128, C], [8, O], [1, D]]),
                    )
                    nc.vector.tensor_copy(
                        wpre[:, rq],
                        wtmp[:].rearrange("p c o i -> p i c o"),
                    )
                    nc.vector.tensor_reduce(
                        ws2[:, rq * D:(rq + 1) * D, :],
                        wtmp[:].rearrange("p c o i -> p i c o"),
                        axis=mybir.AxisListType.X, op=ALU.add,
                    )

                for blk in range(B // PCB):
                    for par in range(2):
                        nc.scalar.activation(
                            uTpre[:, par * 36:(par + 1) * 36, blk * PCB:(blk + 1) * PCB]
                            .rearrange("p q b -> p b q"),
                            psums[(blk, par)][:],
                            AF.Identity,
                            bias=(pcb0 if par == 0 else pcb1)[:],
                            scale=1.0,
                        )
                pcps_ctx.__exit__(None, None, None)

            # ---------------- squash over capsule dim ----------------
            sq = sp.tile([128, Q, B], F32)
            nc.vector.tensor_tensor(sq[:], uTpre[:], uTpre[:], op=ALU.mult)
            sn = sp.tile([128, RQ, B], F32)
            nc.vector.tensor_reduce(
                sn[:], sq[:].rearrange("p (rq i) b -> p rq b i", i=D),
                axis=mybir.AxisListType.X, op=ALU.add,
            )
            t1 = sp.tile([128, RQ, B], F32)
            nc.vector.tensor_scalar_add(t1[:], sn[:], 1.0)
            t2 = sp.tile([128, RQ, B], F32)
            nc.scalar.activation(t2[:], sn[:], AF.Sqrt)
            nc.vector.tensor_scalar_add(t2[:], t2[:], EPS)
            nc.vector.tensor_tensor(t1[:], t1[:], t2[:], op=ALU.mult)
            t3 = sp.tile([128, RQ, B], F32)
            nc.vector.reciprocal(t3[:], t1[:])
            nc.vector.tensor_tensor(t3[:], sn[:], t3[:], op=ALU.mult)  # coef
            nc.vector.tensor_tensor(
                uT[:].rearrange("p (rq i) b -> p rq i b", i=D),
                uTpre[:].rearrange("p (rq i) b -> p rq i b", i=D),
                _ap(t3[:], 0, [t3[:].ap[0], [B, RQ], [0, D], [1, B]]),
                op=ALU.mult,
            )

            # ---------------- routing ----------------
            with (
                tc.tile_pool(name="route", bufs=1) as rp,
                tc.tile_pool(name="rps", bufs=1, space="PSUM") as rps,
            ):
                bij = [rp.tile([80, RQ, 128], F32, name=f"bij{i}_" + sfx)
                       for i in range(2)]
                # u2[b, q, p] = u[b, 9p+q//8, q%8]
                u2 = rp.tile([B, Q, 128], BF16, name="u2_" + sfx)
                bupd = rp.tile([128, RQ, CO], BF16, name="bupd" + sfx)
                # u2[b,q,p] = uT[p,q,b] via PE transposes; copies spread
                # across scalar/vector/gpsimd so no one engine serializes
                for q in range(Q):
                    tps = rps.tile([B, 128], BF16, tag="ut", bufs=2)
                    nc.tensor.transpose(tps[:], uT[:, q, :], ident128[:])
                    if q % 3 == 0:
                        nc.scalar.copy(u2[:, q, :], tps[:])
                    elif q % 3 == 1:
                        nc.vector.tensor_copy(u2[:, q, :], tps[:])
                    else:
                        nc.gpsimd.tensor_copy(u2[:, q, :], tps[:])

                for it in range(3):
                    cbf = []
                    if it > 0:
                        for hf in range(2):
                            bt = bij[hf]
                            btf = bt[:].rearrange("co rq p -> co (rq p)")
                            mx = rp.tile([80, 1], F32, tag="mx", bufs=4)
                            nc.vector.tensor_reduce(
                                mx[:], btf, axis=mybir.AxisListType.X,
                                op=ALU.max, negate=True,
                            )
                            cb_t = rp.tile([80, RQ, 128], BF16, tag=f"c{hf}",
                                           bufs=2)
                            sm = rp.tile([80, 1], F32, tag="sm", bufs=4)
                            nc.scalar.activation(
                                cb_t[:].rearrange("co rq p -> co (rq p)"), btf,
                                AF.Exp, bias=mx[:], scale=1.0,
                                accum_out=sm[:],
                            )
                            rc = rp.tile([80, 1], F32, tag="rc", bufs=4)
                            nc.vector.reciprocal(rc[:], sm[:])
                            nc.vector.tensor_scalar_mul(
                                cb_t[:].rearrange("co rq p -> co (rq p)"),
                                cb_t[:].rearrange("co rq p -> co (rq p)"), rc[:])
                            cbf.append(cb_t)

                    # s matmuls; Wc built from pre-staged wpre (it>0 only)
                    sps = rps.tile([B, CO], F32, tag="s", bufs=1)
                    for rq in range(RQ):
                        if it == 0:
                            wmm = wpre[:, rq]
                        else:
                            wcs = rp.tile([128, D, C, O], BF16, tag="wcs", bufs=3)
                            for hf in range(2):
                                ctp = rps.tile([128, 80], BF16, tag="ct", bufs=1)
                                nc.tensor.transpose(
                                    ctp[:], cbf[hf][:, rq, :], ident128[:80, :80]
                                )
                                weng = nc.vector if hf == 0 else nc.gpsimd
                                weng.tensor_tensor(
                                    wcs[:, :, hf * 5:(hf + 1) * 5, :],
                                    _ap(ctp[:], 0,
                                        [ctp[:].ap[0], [0, D], [16, C // 2], [1, O]]),
                                    wpre[:, rq, :, hf * 5:(hf + 1) * 5, :],
                                    op=ALU.mult,
                                )
                            wmm = wcs[:]
                        for i in range(D):
                            q = rq * D + i
                            nc.tensor.matmul(
                                sps[:], uT[:, q, :],
                                wmm[:, i].rearrange("p c o -> p (c o)"),
                                start=(q == 0), stop=(q == Q - 1),
                            )

                    ssb = rp.tile([B, CO], F32, tag="ssb", bufs=2)
                    nc.scalar.activation(
                        ssb[:], sps[:], AF.Copy,
                        scale=(1.0 / R) if it == 0 else 1.0,
                    )
                    # elementwise squash -> v
                    sa = rp.tile([B, CO], F32, tag="sa", bufs=2)
                    nc.vector.tensor_tensor(sa[:], ssb[:], ssb[:], op=ALU.mult)
                    sb_ = rp.tile([B, CO], F32, tag="sb_", bufs=2)
                    nc.scalar.activation(sb_[:], sa[:], AF.Sqrt)
                    nc.vector.tensor_scalar_add(sb_[:], sb_[:], EPS)
                    sc_ = rp.tile([B, CO], F32, tag="sc_", bufs=2)
                    nc.vector.tensor_scalar_add(sc_[:], sa[:], 1.0)
                    nc.vector.tensor_tensor(sb_[:], sb_[:], sc_[:], op=ALU.mult)
                    nc.vector.reciprocal(sb_[:], sb_[:])
                    nc.vector.tensor_tensor(sa[:], sa[:], sb_[:], op=ALU.mult)  # coef
                    if it == 2:
                        vout = rp.tile([B, CO], F32, tag="vout")
                        nc.vector.tensor_tensor(vout[:], ssb[:], sa[:], op=ALU.mult)
                        nc.sync.dma_start(
                            out_d.ap().rearrange("b c o -> b (c o)"), vout[:]
                        )
                    else:
                        vbf = rp.tile([B, CO], BF16, tag="vbf", bufs=2)
                        nc.vector.tensor_tensor(vbf[:], ssb[:], sa[:], op=ALU.mult)
                        for rq in range(RQ):
                            # G in 3 bank-sized psum pieces (matmul outputs
                            # must not cross the 2KB psum bank boundary)
                            pieces = [(0, 3), (3, 3), (6, 2)]
                            gtiles = []
                            for k, (_, n) in enumerate(pieces):
                                gt = rps.tile([128, n, CO], F32, tag=f"g{k}",
                                              bufs=1, name=f"gps{k}_" + sfx)
                                gtiles.append(gt)
                            for i in range(D):
                                q = rq * D + i
                                k = 0 if i < 3 else (1 if i < 6 else 2)
                                i0 = pieces[k][0]
                                nc.tensor.matmul(
                                    gtiles[k][:, i - i0, :],
                                    u2[:, q, :], vbf[:],
                                    start=True, stop=True,
                                )
                            # gw[p,i,c,o] = G[p,i,c,o]/denom * ws2[p,(rq,i),c]
                            gw = rp.tile([128, D, C, O], F32, tag="gw", bufs=2)
                            for k, (i0, n) in enumerate(pieces):
                                ws_bc = _ap(
                                    ws2[:, rq * D + i0, :], 0,
                                    [ws2[:, 0, :].ap[0], [C, n], [1, C], [0, O]])
                                seng = nc.vector if (rq + k) % 2 == 0 else nc.gpsimd
                                seng.scalar_tensor_tensor(
                                    gw[:, i0:i0 + n],
                                    gtiles[k][:].rearrange(
                                        "p i (c o) -> p i c o", o=O),
                                    1.0 / denom, ws_bc,
                                    op0=ALU.mult, op1=ALU.mult,
                                )
                            # sum over i via contiguous binary tree (gpsimd,
                            # freeing DVE for the next rq's gw multiply)
                            teng = nc.gpsimd if rq % 2 == 0 else nc.vector
                            gh = rp.tile([128, 4, C, O], F32, tag="gh", bufs=2)
                            teng.tensor_tensor(
                                gh[:], gw[:, 0:4], gw[:, 4:8], op=ALU.add)
                            gq = rp.tile([128, 2, C, O], F32, tag="gq", bufs=2)
                            teng.tensor_tensor(
                                gq[:], gh[:, 0:2], gh[:, 2:4], op=ALU.add)
                            teng.tensor_tensor(
                                bupd[:, rq, :].rearrange("p (c o) -> p c o", o=O),
                                gq[:, 0], gq[:, 1], op=ALU.add)
                        arin = dp.tile([128, RQ * CO], BF16,
                                       tag=f"arin{it}_" + sfx)
                        arout = dp.tile([128, RQ * CO], BF16,
                                        tag=f"arout{it}_" + sfx,
                                        addr_space="Shared")
                        nc.sync.dma_start(
                            arin, bupd[:].rearrange("p rq co -> p (rq co)")
                        )
                        if sim_mode:
                            nc.sync.dma_start(arout, arin)
                        else:
                            nc.gpsimd.collective_compute(
                                "AllReduce", ALU.add,
                                replica_groups=[list(range(N_CORES))],
                                ins=[arin.opt()], outs=[arout.opt()],
                            )
                        art = rp.tile([128, RQ, CO], BF16, tag="art", bufs=2)
                        nc.sync.dma_start(
                            art[:].rearrange("p rq co -> p (rq co)"), arout
                        )
                        # transpose (p,(rq,co)) -> bij[co, r=9p+rq] on PE
                        for hf in range(2):
                            for rq in range(RQ):
                                btp = rps.tile([80, 128], BF16, tag="bt", bufs=1)
                                nc.tensor.transpose(
                                    btp[:], art[:, rq, hf * 80:(hf + 1) * 80],
                                    ident128[:],
                                )
                                if it == 0:
                                    nc.vector.tensor_copy(bij[hf][:, rq, :], btp[:])
                                else:
                                    nc.vector.tensor_tensor(
                                        bij[hf][:, rq, :], bij[hf][:, rq, :],
                                        btp[:], op=ALU.add,
                                    )

_NC = None
_WHASH = None


def _weights(inputs):
    return {
        k: np.ascontiguousarray(inputs[k], dtype=np.float32)
        for k in ("conv_w", "conv_b", "pc_w", "pc_b", "W")
    }


def _whash(w):
    h = hashlib.sha256()
    for k in ("conv_w", "conv_b", "pc_w", "pc_b", "W"):
        h.update(w[k].tobytes())
    return h.hexdigest()


def _ensure_built(inputs):
    global _NC, _WHASH
    w = _weights(inputs)
    hsh = _whash(w)
    if _NC is None or hsh != _WHASH:
        _NC = _build(w)
        _WHASH = hsh
    return _NC


def _in_maps(inputs):
    x = np.ascontiguousarray(inputs["x"], dtype=np.float32)
    return [{"x": x[c * B:(c + 1) * B]} for c in range(N_CORES)]


def kernel(**inputs):
    nc = _ensure_built(inputs)
    res = run_bass_kernel_spmd(nc, _in_maps(inputs), core_ids=list(range(N_CORES)))
    return np.concatenate([res.results[c]["out"] for c in range(N_CORES)], axis=0)


def _pipelined_mins(nc, in_maps, ns, n_trials):
    import time
    import jax
    from jax.sharding import Mesh, PartitionSpec
    from jax.experimental.shard_map import shard_map
    import concourse.bass2jax as b2j
    import concourse.mybir as mybir_

    b2j.install_neuronx_cc_hook()
    partition_name = nc.partition_id_tensor.name if nc.partition_id_tensor else None
    in_names, out_names, out_avals, zero_outs = [], [], [], []
    for alloc in nc.m.functions[0].allocations:
        if not isinstance(alloc, mybir_.MemoryLocationSet):
            continue
        name = alloc.memorylocations[0].name
        if alloc.kind == "ExternalInput":
            if name != partition_name:
                in_names.append(name)
        elif alloc.kind == "ExternalOutput":
            shape = tuple(alloc.tensor_shape)
            dtype = mybir_.dt.np(alloc.dtype)
            out_names.append(name)
            out_avals.append(jax.core.ShapedArray(shape, dtype))
            zero_outs.append(np.zeros(shape, dtype))
    n_params = len(in_names)
    n_outs = len(out_avals)
    all_in_names = list(in_names) + out_names
    if partition_name is not None:
        all_in_names.append(partition_name)
    donate = tuple(range(n_params, n_params + n_outs))

    def _body(*args):
        operands = list(args)
        if partition_name is not None:
            operands.append(b2j.partition_id_tensor())
        outs = b2j._bass_exec_p.bind(
            *operands,
            out_avals=tuple(out_avals),
            in_names=tuple(all_in_names),
            out_names=tuple(out_names),
            lowering_input_output_aliases=(),
            sim_require_finite=True,
            sim_require_nnan=True,
            nc=nc,
        )
        return tuple(outs)

    devices = jax.devices()[:N_CORES]
    mesh = Mesh(np.asarray(devices), ("core",))
    in_specs = (PartitionSpec("core"),) * (n_params + n_outs)
    out_specs = (PartitionSpec("core"),) * n_outs
    sharded = jax.jit(
        shard_map(_body, mesh=mesh, in_specs=in_specs, out_specs=out_specs,
                  check_rep=False),
        donate_argnums=donate, keep_unused=True,
    )
    concat_in = [
        jax.device_put(
            np.concatenate([np.asarray(in_maps[c][n]) for c in range(N_CORES)],
                           axis=0))
        for n in in_names
    ]
    mins = {}
    for n in ns:
        times = []
        for _ in range(n_trials):
            zsets = [
                [jax.device_put(
                    np.zeros((N_CORES * z.shape[0], *z.shape[1:]), z.dtype))
                 for z in zero_outs]
                for _ in range(n)
            ]
            jax.block_until_ready(zsets)
            t0 = time.perf_counter()
            outs = [sharded(*concat_in, *zsets[k]) for k in range(n)]
            jax.block_until_ready(outs)
            times.append((time.perf_counter() - t0) * 1e9)
        mins[n] = min(times)
        print(f"  N={n} pipelined dispatches, wall (ms): "
              + ", ".join(f"{t/1e6:.2f}" for t in times))
    return mins


_TIMING_REPS = 8


def run_timed(**inputs):
    """Measure per-forward HW time of the compiled kernel.

    The axon NTFF profiling hook is unavailable here, so there is no
    hardware profile to read. Wall-clock around one dispatch is useless:
    a jitted `x+1` costs the same ~60 ms tunnel round-trip, and even the
    marginal cost of an extra pipelined dispatch is ~3-4 ms of dispatch
    overhead for an empty kernel. Instead, build a second NEFF whose body
    is the SAME forward pass replicated _TIMING_REPS times back-to-back,
    measure the marginal cost per pipelined dispatch of both NEFFs, and
    difference them:

        slope(K) - slope(1) = (K - 1) * t_body

    which cancels both the tunnel round-trip and the per-dispatch
    overhead, leaving the steady-state device-side time of one forward
    (conv + routing + 2 AllReduces across the 8 cores).
    """
    nc1 = _ensure_built(inputs)
    ncK = _build(_weights(inputs), reps=_TIMING_REPS)
    in_maps = _in_maps(inputs)

    ns = (1, 48)
    n1, n2 = ns
    print(f"[timing] reps=1 NEFF:")
    m1 = _pipelined_mins(nc1, in_maps, ns, n_trials=12)
    print(f"[timing] reps={_TIMING_REPS} NEFF:")
    mK = _pipelined_mins(ncK, in_maps, ns, n_trials=12)
    slope1 = (m1[n2] - m1[n1]) / (n2 - n1)
    slopeK = (mK[n2] - mK[n1]) / (n2 - n1)
    body = (slopeK - slope1) / (_TIMING_REPS - 1)
    print(f"single-dispatch wall: {m1[n1]/1e6:.3f} ms; "
          f"marginal/dispatch reps=1: {slope1/1e6:.4f} ms; "
          f"reps={_TIMING_REPS}: {slopeK/1e6:.4f} ms; "
          f"per-forward body: {body/1e6:.4f} ms")
    return int(max(body, 1.0))


# revision 53
# speedup vs baseline: 1.3591x; 1.3453x over previous
"""CapsNet forward kernel for 8 TRN2 NeuronCores (data-parallel over batch).

Per core (b=32 local batch):
  h  = relu(conv(x, conv_w, s1)+cb)            (b,256,20,20)
  u  = squash_8(conv(h, pc_w, s2)+pb)          (b,1152,8)
  routing without materializing u_hat:
    s_k[b,co]   = sum_{p,q} Wc_k[p,q,co] * uT[p,q,b]      (PE)
    v_k         = elementwise-squash(s_k)
    b_upd[r,co] = sum_i Ws[r,c,i] * (1/B sum_b u[b,ri] v[b,co])
    b_ij += AllReduce(b_upd); c = softmax_r(b_ij); Wc = c * W
  Weights (conv_w, conv_b, pc_w, pc_b, W) are identical on every core and
  known at build time, so they are embedded in the NEFF as Const DRAM
  tensors (loaded to HBM once at model-load) instead of per-call inputs.

Partition p (0..127) is an out-channel PAIR (oc=2p / 2p+1), q (0..71) is
(oc%2)*36 + yx; global route r = 9p + q//8, capsule elem i = q%8.
"""
import hashlib

import numpy as np

import concourse.bass as bass
import concourse.mybir as mybir
import concourse.tile as tile
from concourse import bacc
from concourse.bass_utils import run_bass_kernel_spmd
from concourse.masks import make_identity

F32 = mybir.dt.float32
F32R = mybir.dt.float32r
BF16 = mybir.dt.bfloat16
AF = mybir.ActivationFunctionType
ALU = mybir.AluOpType

N_CORES = 8
B = 32              # per-core batch
C1B = 4             # conv1 batch block
PCB = 8             # pc-conv batch block
R, C, O, D = 1152, 10, 16, 8
CO = C * O
Q = 72
RQ = 9
EPS = 1e-5


def _ap(t, offset, dims):
    return bass.AP(t.tensor, t.offset + offset, dims)


def _build(weights, sim_mode=False, reps=1):
    ncores = 1 if sim_mode else N_CORES
    # sim bypasses the AllReduce, so the batch-mean denominator is local-only
    denom = float(B * (1 if sim_mode else N_CORES))
    nc = bacc.Bacc("TRN2", target_bir_lowering=False, debug=False, num_devices=ncores)

    x_d = nc.dram_tensor("x", [B, 1, 28, 28], F32, kind="ExternalInput")
    cw_d = nc.inline_tensor(weights["conv_w"], name="conv_w")
    cb_d = nc.inline_tensor(weights["conv_b"], name="conv_b")
    pw_d = nc.inline_tensor(weights["pc_w"], name="pc_w")
    pb_d = nc.inline_tensor(weights["pc_b"], name="pc_b")
    w_d = nc.inline_tensor(weights["W"], name="W")
    out_d = nc.dram_tensor("out", [B, C, O], F32, kind="ExternalOutput")

    with tile.TileContext(nc) as tc:
        for _rep in range(reps):
            _fwd(nc, tc, x_d, cw_d, cb_d, pw_d, pb_d, w_d, out_d,
                 denom, sim_mode, str(_rep))

    nc.compile()
    return nc


def _fwd(nc, tc, x_d, cw_d, cb_d, pw_d, pb_d, w_d, out_d, denom, sim_mode, sfx):
    with (
        tc.tile_pool(name="persist" + sfx, bufs=1) as pp,
        tc.tile_pool(name="small" + sfx, bufs=1) as sp,
        tc.tile_pool(name="dram" + sfx, bufs=1, space="DRAM") as dp,
    ):
        if True:
            # ---------------- small loads ----------------
            # cw1T partition p = kx*9 + ky (matches xs tile layout below)
            cw1T = pp.tile([81, 256], F32)
            for kx in range(9):
                nc.sync.dma_start(
                    cw1T[kx * 9:(kx + 1) * 9, :],
                    bass.AP(cw_d, kx, [[9, 9], [81, 256]]),
                )
            cw1bf = pp.tile([81, 256], BF16)
            nc.vector.tensor_copy(cw1bf[:], cw1T[:])
            cb0 = sp.tile([128, 1], F32)
            cb1 = sp.tile([128, 1], F32)
            cbv = cb_d.ap().rearrange("(a b) -> a b", b=1)
            nc.sync.dma_start(cb0[:], cbv[0:128])
            nc.sync.dma_start(cb1[:], cbv[128:256])
            pcb0 = sp.tile([128, 1], F32)
            pcb1 = sp.tile([128, 1], F32)
            pbv = pb_d.ap().rearrange("(p two) -> p two", two=2)
            nc.sync.dma_start(pcb0[:], pbv[:, 0:1])
            nc.sync.dma_start(pcb1[:], pbv[:, 1:2])
            ident128 = pp.tile([128, 128], BF16)
            make_identity(nc, ident128[:])
            # x -> bf16, padded to 792/image, staged in DRAM; then one
            # HBM->HBM 4D im2col so the SBUF load is a single full-width DMA
            with tc.tile_pool(name="xprep", bufs=1) as xp:
                xsb = xp.tile([B, 784], F32)
                nc.sync.dma_start(
                    xsb[:], x_d.ap().rearrange("b one h w -> b (one h w)"))
                xbf = xp.tile([B, 792], BF16)
                nc.vector.memset(xbf[:, 784:], 0.0)
                nc.vector.tensor_copy(xbf[:, 0:784], xsb[:])
                xpad_d = dp.tile([B, 792], BF16, tag="xpad" + sfx)
                nc.sync.dma_start(xpad_d[:], xbf[:])
            xs2_d = dp.tile([81, B, 560], BF16, tag="xs2" + sfx)
            for kx in range(9):
                nc.sync.dma_start(
                    _ap(xs2_d, kx * 9 * B * 560,
                        [[B * 560, 9], [560, B], [1, 560]]),
                    _ap(xpad_d, kx, [[28, 9], [792, B], [1, 560]]),
                )

            h_sb = [pp.tile([128, B, 20, 20], BF16, name=f"h{i}_" + sfx) for i in range(2)]
            uTpre = pp.tile([128, Q, B], F32)
            uT = pp.tile([128, Q, B], BF16)

            ws2 = pp.tile([128, Q, C], F32)

            # -------- conv1 + primary-caps conv (pc_w ic0 loads hoisted) -----
            with tc.tile_pool(name="pcw", bufs=1) as pwp:
                def load_pwin(ic_t, par):
                    pwin = pwp.tile([128, 128, 81], BF16, tag="pwin", bufs=2)
                    for sub in range(4):
                        tmp = pwp.tile([128, 32, 81], F32, tag="pwt", bufs=2)
                        src = bass.AP(
                            pw_d,
                            (2 * sub * 32 + par) * 20736 + ic_t * 128 * 81,
                            [[81, 128], [2 * 20736, 32], [1, 81]],
                        )
                        nc.sync.dma_start(tmp[:], src)
                        eng = nc.vector if sub % 2 == 0 else nc.gpsimd
                        eng.tensor_copy(
                            pwin[:, sub * 32:(sub + 1) * 32, :], tmp[:])
                    return pwin

                with (
                    tc.tile_pool(name="c1in", bufs=1) as c1p,
                    tc.tile_pool(name="c1ps", bufs=1, space="PSUM") as c1ps,
                ):
                    # partition p = kx*9 + ky; one contiguous full-width load,
                    # queued BEFORE the pc_w loads so conv1 starts early
                    xs_all = c1p.tile([81, B, 560], BF16, tag="xsall")
                    nc.sync.dma_start(
                        xs_all[:].rearrange("p b c -> p (b c)"),
                        _ap(xs2_d, 0, [[B * 560, 81], [1, B * 560]]),
                    )
                    pwins = {(0, 0): load_pwin(0, 0), (0, 1): load_pwin(0, 1)}
                    for bb in range(B):
                        for half in range(2):
                            for oct_ in range(2):
                                ps = c1ps.tile([128, 280], F32, tag="c1", bufs=7)
                                lhsT = cw1bf[:, oct_ * 128:(oct_ + 1) * 128]
                                rhs = xs_all[:, bb, half * 280: half * 280 + 280]
                                nc.tensor.matmul(ps[:], lhsT, rhs, start=True, stop=True)
                                hd = h_sb[oct_][:, bb, half * 10:(half + 1) * 10, :]
                                pv = ps[:].rearrange("p (y x) -> p y x", x=28)[:, :, 0:20]
                                cbx = (cb0 if oct_ == 0 else cb1)
                                if (bb * 4 + half * 2 + oct_) % 2 == 0:
                                    nc.scalar.activation(
                                        hd, pv, AF.Relu, bias=cbx[:], scale=1.0)
                                else:
                                    # bias-add + relu on DVE to split the
                                    # epilogue across both engines
                                    nc.vector.tensor_scalar(
                                        hd, pv, cbx[:], 0.0,
                                        op0=ALU.add, op1=ALU.max)

                pcps_ctx = tc.tile_pool(name="pcps", bufs=1, space="PSUM")
                pcps = pcps_ctx.__enter__()
                psums = {}
                for blk in range(B // PCB):
                    for par in range(2):
                        psums[(blk, par)] = pcps.tile(
                            [128, PCB, 36], F32, tag=f"pc{blk}{par}", bufs=1,
                            name=f"pcps{blk}{par}_" + sfx,
                        )
                for ic_t in range(2):
                    for par in range(2):
                        pwin = pwins.get((ic_t, par))
                        if pwin is None:
                            pwin = load_pwin(ic_t, par)
                        for t in range(81):
                            ky, kx = t // 9, t % 9
                            for blk in range(B // PCB):
                                rhs = h_sb[ic_t][:, blk * PCB:(blk + 1) * PCB,
                                                 ky:ky + 12:2, kx:kx + 12:2]
                                nc.tensor.matmul(
                                    psums[(blk, par)][:], pwin[:, :, t], rhs,
                                    start=(ic_t == 0 and t == 0),
                                    stop=(ic_t == 1 and t == 80),
                                )
                # One-time W pre-stage (issued here so the W DMAs queue
                # behind the pc_w loads and the DVE transforms run during the
                # pc-conv matmuls instead of blocking the routing phase).
                wpre = pp.tile([128, RQ, D, C, O], BF16, name="wpre" + sfx)
                for rq in range(RQ):
                    wtmp = pwp.tile([128, C, O, D], F32, tag="wtmp", bufs=1)
                    nc.sync.dma_start(
                        wtmp[:],
                        bass.AP(w_d, rq * 1280,
                                [[RQ * 1280, 128], [128, C], [8, O], [1, D]]),
                    )
                    nc.vector.tensor_copy(
                        wpre[:, rq],
                        wtmp[:].rearrange("p c o i -> p i c o"),
                    )
                    nc.vector.tensor_reduce(
                        ws2[:, rq * D:(rq + 1) * D, :],
                        wtmp[:].rearrange("p c o i -> p i c o"),
                        axis=mybir.AxisListType.X, op=ALU.add,
                    )

                for blk in range(B // PCB):
                    for par in range(2):
                        nc.scalar.activation(
                            uTpre[:, par * 36:(par + 1) * 36, blk * PCB:(blk + 1) * PCB]
                            .rearrange("p q b -> p b q"),
                            psums[(blk, par)][:],
                            AF.Identity,
                            bias=(pcb0 if par == 0 else pcb1)[:],
                            scale=1.0,
                        )
                pcps_ctx.__exit__(None, None, None)

            # ---------------- squash over capsule dim ----------------
            sq = sp.tile([128, Q, B], F32)
            nc.vector.tensor_tensor(sq[:], uTpre[:], uTpre[:], op=ALU.mult)
            sn = sp.tile([128, RQ, B], F32)
            nc.vector.tensor_reduce(
                sn[:], sq[:].rearrange("p (rq i) b -> p rq b i", i=D),
                axis=mybir.AxisListType.X, op=ALU.add,
            )
            t1 = sp.tile([128, RQ, B], F32)
            nc.vector.tensor_scalar_add(t1[:], sn[:], 1.0)
            t2 = sp.tile([128, RQ, B], F32)
            nc.scalar.activation(t2[:], sn[:], AF.Sqrt)
            nc.vector.tensor_scalar_add(t2[:], t2[:], EPS)
            nc.vector.tensor_tensor(t1[:], t1[:], t2[:], op=ALU.mult)
            t3 = sp.tile([128, RQ, B], F32)
            nc.vector.reciprocal(t3[:], t1[:])
            nc.vector.tensor_tensor(t3[:], sn[:], t3[:], op=ALU.mult)  # coef
            nc.vector.tensor_tensor(
                uT[:].rearrange("p (rq i) b -> p rq i b", i=D),
                uTpre[:].rearrange("p (rq i) b -> p rq i b", i=D),
                _ap(t3[:], 0, [t3[:].ap[0], [B, RQ], [0, D], [1, B]]),
                op=ALU.mult,
            )

            # ---------------- routing ----------------
            with (
                tc.tile_pool(name="route", bufs=1) as rp,
                tc.tile_pool(name="rps", bufs=1, space="PSUM") as rps,
            ):
                bij = [rp.tile([80, RQ, 128], F32, name=f"bij{i}_" + sfx)
                       for i in range(2)]
                # u2[b, q, p] = u[b, 9p+q//8, q%8]
                u2 = rp.tile([B, Q, 128], BF16, name="u2_" + sfx)
                bupd = rp.tile([128, RQ, CO], BF16, name="bupd" + sfx)
                bupdT = rp.tile([80, 2, RQ, 128], BF16, name="bupdT" + sfx)
                # u2[b,q,p] = uT[p,q,b] via PE transposes; copies spread
                # across scalar/vector/gpsimd so no one engine serializes
                for q in range(Q):
                    tps = rps.tile([B, 128], BF16, tag="ut", bufs=2)
                    nc.tensor.transpose(tps[:], uT[:, q, :], ident128[:])
                    if q % 3 == 0:
                        nc.scalar.copy(u2[:, q, :], tps[:])
                    elif q % 3 == 1:
                        nc.vector.tensor_copy(u2[:, q, :], tps[:])
                    else:
                        nc.gpsimd.tensor_copy(u2[:, q, :], tps[:])

                for it in range(3):
                    cbf = []
                    if it > 0:
                        for hf in range(2):
                            bt = bij[hf]
                            btf = bt[:].rearrange("co rq p -> co (rq p)")
                            mx = rp.tile([80, 1], F32, tag="mx", bufs=4)
                            nc.vector.tensor_reduce(
                                mx[:], btf, axis=mybir.AxisListType.X,
                                op=ALU.max, negate=True,
                            )
                            cb_t = rp.tile([80, RQ, 128], BF16, tag=f"c{hf}",
                                           bufs=2)
                            sm = rp.tile([80, 1], F32, tag="sm", bufs=4)
                            nc.scalar.activation(
                                cb_t[:].rearrange("co rq p -> co (rq p)"), btf,
                                AF.Exp, bias=mx[:], scale=1.0,
                                accum_out=sm[:],
                            )
                            rc = rp.tile([80, 1], F32, tag="rc", bufs=4)
                            nc.vector.reciprocal(rc[:], sm[:])
                            nc.vector.tensor_scalar_mul(
                                cb_t[:].rearrange("co rq p -> co (rq p)"),
                                cb_t[:].rearrange("co rq p -> co (rq p)"), rc[:])
                            cbf.append(cb_t)

                    # s matmuls; Wc built from pre-staged wpre (it>0 only)
                    sps = rps.tile([B, CO], F32, tag="s", bufs=1)
                    for rq in range(RQ):
                        if it == 0:
                            wmm = wpre[:, rq]
                        else:
                            wcs = rp.tile([128, D, C, O], BF16, tag="wcs", bufs=3)
                            for hf in range(2):
                                ctp = rps.tile([128, 80], BF16, tag="ct", bufs=1)
                                nc.tensor.transpose(
                                    ctp[:], cbf[hf][:, rq, :], ident128[:80, :80]
                                )
                                weng = nc.vector if hf == 0 else nc.gpsimd
                                weng.tensor_tensor(
                                    wcs[:, :, hf * 5:(hf + 1) * 5, :],
                                    _ap(ctp[:], 0,
                                        [ctp[:].ap[0], [0, D], [16, C // 2], [1, O]]),
                                    wpre[:, rq, :, hf * 5:(hf + 1) * 5, :],
                                    op=ALU.mult,
                                )
                            wmm = wcs[:]
                        for i in range(D):
                            q = rq * D + i
                            nc.tensor.matmul(
                                sps[:], uT[:, q, :],
                                wmm[:, i].rearrange("p c o -> p (c o)"),
                                start=(q == 0), stop=(q == Q - 1),
                            )

                    ssb = rp.tile([B, CO], F32, tag="ssb", bufs=2)
                    nc.scalar.activation(
                        ssb[:], sps[:], AF.Copy,
                        scale=(1.0 / R) if it == 0 else 1.0,
                    )
                    # elementwise squash -> v
                    sa = rp.tile([B, CO], F32, tag="sa", bufs=2)
                    nc.vector.tensor_tensor(sa[:], ssb[:], ssb[:], op=ALU.mult)
                    sb_ = rp.tile([B, CO], F32, tag="sb_", bufs=2)
                    nc.scalar.activation(sb_[:], sa[:], AF.Sqrt)
                    nc.vector.tensor_scalar_add(sb_[:], sb_[:], EPS)
                    sc_ = rp.tile([B, CO], F32, tag="sc_", bufs=2)
                    nc.vector.tensor_scalar_add(sc_[:], sa[:], 1.0)
                    nc.vector.tensor_tensor(sb_[:], sb_[:], sc_[:], op=ALU.mult)
                    nc.vector.reciprocal(sb_[:], sb_[:])
                    nc.vector.tensor_tensor(sa[:], sa[:], sb_[:], op=ALU.mult)  # coef
                    if it == 2:
                        vout = rp.tile([B, CO], F32, tag="vout")
                        nc.vector.tensor_tensor(vout[:], ssb[:], sa[:], op=ALU.mult)
                        nc.sync.dma_start(
                            out_d.ap().rearrange("b c o -> b (c o)"), vout[:]
                        )
                    else:
                        vbf = rp.tile([B, CO], BF16, tag="vbf", bufs=2)
                        nc.vector.tensor_tensor(vbf[:], ssb[:], sa[:], op=ALU.mult)
                        for rq in range(RQ):
                            # G in 3 bank-sized psum pieces (matmul outputs
                            # must not cross the 2KB psum bank boundary)
                            pieces = [(0, 3), (3, 3), (6, 2)]
                            gtiles = []
                            for k, (_, n) in enumerate(pieces):
                                gt = rps.tile([128, n, CO], F32, tag=f"g{k}",
                                              bufs=1, name=f"gps{k}_" + sfx)
                                gtiles.append(gt)
                            for i in range(D):
                                q = rq * D + i
                                k = 0 if i < 3 else (1 if i < 6 else 2)
                                i0 = pieces[k][0]
                                nc.tensor.matmul(
                                    gtiles[k][:, i - i0, :],
                                    u2[:, q, :], vbf[:],
                                    start=True, stop=True,
                                )
                            # gw[p,i,c,o] = G[p,i,c,o]/denom * ws2[p,(rq,i),c]
                            gw = rp.tile([128, D, C, O], F32, tag="gw", bufs=2)
                            for k, (i0, n) in enumerate(pieces):
                                ws_bc = _ap(
                                    ws2[:, rq * D + i0, :], 0,
                                    [ws2[:, 0, :].ap[0], [C, n], [1, C], [0, O]])
                                seng = nc.vector if (rq + k) % 2 == 0 else nc.gpsimd
                                seng.scalar_tensor_tensor(
                                    gw[:, i0:i0 + n],
                                    gtiles[k][:].rearrange(
                                        "p i (c o) -> p i c o", o=O),
                                    1.0 / denom, ws_bc,
                                    op0=ALU.mult, op1=ALU.mult,
                                )
                            # sum over i via contiguous binary tree (gpsimd,
                            # freeing DVE for the next rq's gw multiply)
                            teng = nc.gpsimd if rq % 2 == 0 else nc.vector
                            gh = rp.tile([128, 4, C, O], F32, tag="gh", bufs=2)
                            teng.tensor_tensor(
                                gh[:], gw[:, 0:4], gw[:, 4:8], op=ALU.add)
                            gq = rp.tile([128, 2, C, O], F32, tag="gq", bufs=2)
                            teng.tensor_tensor(
                                gq[:], gh[:, 0:2], gh[:, 2:4], op=ALU.add)
                            teng.tensor_tensor(
                                bupd[:, rq, :].rearrange("p (c o) -> p c o", o=O),
                                gq[:, 0], gq[:, 1], op=ALU.add)
                            # transpose this rq's update to bij layout NOW
                            # (reduce-then-transpose == transpose-then-reduce,
                            # so the PE work hides inside the g-phase and the
                            # post-AllReduce path is just a load + two adds)
                            for hf in range(2):
                                btp = rps.tile([80, 128], BF16, tag="bt",
                                               bufs=1)
                                nc.tensor.transpose(
                                    btp[:], bupd[:, rq, hf * 80:(hf + 1) * 80],
                                    ident128[:],
                                )
                                if hf == 0:
                                    nc.vector.tensor_copy(
                                        bupdT[:, hf, rq, :], btp[:])
                                else:
                                    nc.scalar.copy(
                                        bupdT[:, hf, rq, :], btp[:])
                        arin = dp.tile([80, 2 * RQ * 128], BF16,
                                       tag=f"arin{it}_" + sfx)
                        arout = dp.tile([80, 2 * RQ * 128], BF16,
                                        tag=f"arout{it}_" + sfx,
                                        addr_space="Shared")
                        nc.sync.dma_start(
                            arin,
                            bupdT[:].rearrange("co hf rq p -> co (hf rq p)"),
                        )
                        if sim_mode:
                            nc.sync.dma_start(arout, arin)
                        else:
                            nc.gpsimd.collective_compute(
                                "AllReduce", ALU.add,
                                replica_groups=[list(range(N_CORES))],
                                ins=[arin.opt()], outs=[arout.opt()],
                            )
                        art = rp.tile([80, 2, RQ, 128], BF16, tag="art", bufs=2)
                        nc.sync.dma_start(
                            art[:].rearrange("co hf rq p -> co (hf rq p)"),
                            arout,
                        )
                        for hf in range(2):
                            if it == 0:
                                nc.vector.tensor_copy(bij[hf][:], art[:, hf])
                            else:
                                nc.vector.tensor_tensor(
                                    bij[hf][:], bij[hf][:], art[:, hf],
                                    op=ALU.add,
                                )

_NC = None
_WHASH = None


def _weights(inputs):
    return {
        k: np.ascontiguousarray(inputs[k], dtype=np.float32)
        for k in ("conv_w", "conv_b", "pc_w", "pc_b", "W")
    }


def _whash(w):
    h = hashlib.sha256()
    for k in ("conv_w", "conv_b", "pc_w", "pc_b", "W"):
        h.update(w[k].tobytes())
    return h.hexdigest()


def _ensure_built(inputs):
    global _NC, _WHASH
    w = _weights(inputs)
    hsh = _whash(w)
    if _NC is None or hsh != _WHASH:
        _NC = _build(w)
        _WHASH = hsh
    return _NC


def _in_maps(inputs):
    x = np.ascontiguousarray(inputs["x"], dtype=np.float32)
    return [{"x": x[c * B:(c + 1) * B]} for c in range(N_CORES)]


def kernel(**inputs):
    nc = _ensure_built(inputs)
    res = run_bass_kernel_spmd(nc, _in_maps(inputs), core_ids=list(range(N_CORES)))
    return np.concatenate([res.results[c]["out"] for c in range(N_CORES)], axis=0)


def _pipelined_mins(nc, in_maps, ns, n_trials):
    import time
    import jax
    from jax.sharding import Mesh, PartitionSpec
    from jax.experimental.shard_map import shard_map
    import concourse.bass2jax as b2j
    import concourse.mybir as mybir_

    b2j.install_neuronx_cc_hook()
    partition_name = nc.partition_id_tensor.name if nc.partition_id_tensor else None
    in_names, out_names, out_avals, zero_outs = [], [], [], []
    for alloc in nc.m.functions[0].allocations:
        if not isinstance(alloc, mybir_.MemoryLocationSet):
            continue
        name = alloc.memorylocations[0].name
        if alloc.kind == "ExternalInput":
            if name != partition_name:
                in_names.append(name)
        elif alloc.kind == "ExternalOutput":
            shape = tuple(alloc.tensor_shape)
            dtype = mybir_.dt.np(alloc.dtype)
            out_names.append(name)
            out_avals.append(jax.core.ShapedArray(shape, dtype))
            zero_outs.append(np.zeros(shape, dtype))
    n_params = len(in_names)
    n_outs = len(out_avals)
    all_in_names = list(in_names) + out_names
    if partition_name is not None:
        all_in_names.append(partition_name)
    donate = tuple(range(n_params, n_params + n_outs))

    def _body(*args):
        operands = list(args)
        if partition_name is not None:
            operands.append(b2j.partition_id_tensor())
        outs = b2j._bass_exec_p.bind(
            *operands,
            out_avals=tuple(out_avals),
            in_names=tuple(all_in_names),
            out_names=tuple(out_names),
            lowering_input_output_aliases=(),
            sim_require_finite=True,
            sim_require_nnan=True,
            nc=nc,
        )
        return tuple(outs)

    devices = jax.devices()[:N_CORES]
    mesh = Mesh(np.asarray(devices), ("core",))
    in_specs = (PartitionSpec("core"),) * (n_params + n_outs)
    out_specs = (PartitionSpec("core"),) * n_outs
    sharded = jax.jit(
        shard_map(_body, mesh=mesh, in_specs=in_specs, out_specs=out_specs,
                  check_rep=False),
        donate_argnums=donate, keep_unused=True,
    )
    concat_in = [
        jax.device_put(
            np.concatenate([np.asarray(in_maps[c][n]) for c in range(N_CORES)],
                           axis=0))
        for n in in_names
    ]
    mins = {}
    for n in ns:
        times = []
        for _ in range(n_trials):
            zsets = [
                [jax.device_put(
                    np.zeros((N_CORES * z.shape[0], *z.shape[1:]), z.dtype))
                 for z in zero_outs]
                for _ in range(n)
            ]
            jax.block_until_ready(zsets)
            t0 = time.perf_counter()
            outs = [sharded(*concat_in, *zsets[k]) for k in range(n)]
            jax.block_until_ready(outs)
            times.append((time.perf_counter() - t0) * 1e9)
        mins[n] = min(times)
        print(f"  N={n} pipelined dispatches, wall (ms): "
              + ", ".join(f"{t/1e6:.2f}" for t in times))
    return mins


_TIMING_REPS = 8


def run_timed(**inputs):
    """Measure per-forward HW time of the compiled kernel.

    The axon NTFF profiling hook is unavailable here, so there is no
    hardware profile to read. Wall-clock around one dispatch is useless:
    a jitted `x+1` costs the same ~60 ms tunnel round-trip, and even the
    marginal cost of an extra pipelined dispatch is ~3-4 ms of dispatch
    overhead for an empty kernel. Instead, build a second NEFF whose body
    is the SAME forward pass replicated _TIMING_REPS times back-to-back,
    measure the marginal cost per pipelined dispatch of both NEFFs, and
    difference them:

        slope(K) - slope(1) = (K - 1) * t_body

    which cancels both the tunnel round-trip and the per-dispatch
    overhead, leaving the steady-state device-side time of one forward
    (conv + routing + 2 AllReduces across the 8 cores).
    """
    nc1 = _ensure_built(inputs)
    ncK = _build(_weights(inputs), reps=_TIMING_REPS)
    in_maps = _in_maps(inputs)

    ns = (1, 48)
    n1, n2 = ns
    print(f"[timing] reps=1 NEFF:")
    m1 = _pipelined_mins(nc1, in_maps, ns, n_trials=12)
    print(f"[timing] reps={_TIMING_REPS} NEFF:")
    mK = _pipelined_mins(ncK, in_maps, ns, n_trials=12)
    slope1 = (m1[n2] - m1[n1]) / (n2 - n1)
    slopeK = (mK[n2] - mK[n1]) / (n2 - n1)
    body = (slopeK - slope1) / (_TIMING_REPS - 1)
    print(f"single-dispatch wall: {m1[n1]/1e6:.3f} ms; "
          f"marginal/dispatch reps=1: {slope1/1e6:.4f} ms; "
          f"reps={_TIMING_REPS}: {slopeK/1e6:.4f} ms; "
          f"per-forward body: {body/1e6:.4f} ms")
    return int(max(body, 1.0))
